# revision 1
# baseline (speedup 1.0000x reference)
"""Clockwork RNN (CwRNNCell) Trainium2 Bass kernel.

Reference semantics (T=4096, H=2048, 8 modules of 256, periods 2^j):
  step t (1-indexed): module j active iff t % 2^j == 0
  pre = x_t @ W_ih.T + b_ih + h @ W_hh.T + b_hh
  h[active] = tanh(pre[active]);  inactive modules hold.

Design (single core, everything SBUF/PSUM-resident):
  Stage A (PE, fp32): U = X @ W_ih.T + (b_ih+b_hh), written to DRAM as a
    bf16 hi+lo pair in a (tau,f,p)-swizzled layout so the chain can pull it
    in with 16-bit transpose-DMAs (fp32 transpose-DMA is unsupported).
  Chain: per-step pre-activation tiles live in PSUM (4 banks per 128-step
    superstep, 32 steps x 16 cols per bank) and are built entirely by PE
    accumulation (per-element has_written semantics):
      - an fp32 identity matmul per bank deposits U (start=True opens the
        bank, everything after accumulates),
      - when module m updates at step tau (h_m := tanh output), its column
        product W_hh[:, m-half] @ h_m is deposited into ALL steps of the
        window (tau, tau+2^m] in one broadcast-rhs matmul per output
        row-half (stride-16 psum columns, N = window length),
      - module 0's product (fresh every step) lands as N=1 matmuls into the
        next step's active columns only.
    Consume is a single ScalarE tanh per step (PSUM -> bf16 h in SBUF); DVE
    only stages h into the output buffer (off the critical path).  No DVE
    reduce, no q-slot copies: the serial cycle is tanh -> k0 matmuls -> tanh
    (~0.4-0.6us/step on HW).
  Weights W_hh are bf16 (validated: ~9e-3 absmax err vs fp32 reference).
  U prefetch (transpose-DMA + hi/lo add) and the output path (PE transpose,
  DVE copy, DMA) are double-buffered across supersteps; the hardware loop
  body covers TWO supersteps so ping-pong parity stays static.
"""

import os

import numpy as np
import ml_dtypes

import concourse.bass as bass
import concourse.bacc as bacc
import concourse.mybir as mybir
from concourse import tile
from concourse.bass_utils import run_bass_kernel_spmd

BF16 = mybir.dt.bfloat16
F32 = mybir.dt.float32

H = 2048
IN = 1024
MS = 256
SL = 128  # superstep length (= max period)
BK = 32   # steps per psum bank (512 fp32 cols / 16)


def _ctz(t):
    return (t & -t).bit_length() - 1


def _nu(t):
    return min(_ctz(t), 7)


def _fc(t):
    return 2 * (_nu(t) + 1)


def build_nc(T=4096, num_cores=1, debug=False, enable_asserts=False, repeat=1):
    variant = os.environ.get("KV", "full")
    SS = T // SL
    assert (SS * repeat) % 2 == 0
    nc = bacc.Bacc(
        "TRN2",
        target_bir_lowering=False,
        debug=debug,
        enable_asserts=enable_asserts,
        num_devices=num_cores,
    )

    xT = nc.dram_tensor("xT", [IN, T], F32, kind="ExternalInput").ap()
    wihT = nc.dram_tensor("wihT", [IN, H], F32, kind="ExternalInput").ap()
    whhT = nc.dram_tensor("whhT", [H, H], BF16, kind="ExternalInput").ap()
    bias_bc = nc.dram_tensor("bias_bc", [128, H], F32, kind="ExternalInput").ap()
    ident = nc.dram_tensor("ident", [128, 128], F32, kind="ExternalInput").ap()
    out = nc.dram_tensor("out", [T, H], F32, kind="ExternalOutput").ap()

    # U (hi/lo bf16) in swizzled layout: row (t*16 + f), col p holds
    # U[t, 128*f + p].  One extra superstep of rows so the steady-state
    # prefetch of s+1 needs no modulo wraparound (overrun read unused).
    uhi = nc.dram_tensor("uhi", [(T + SL) * 16, 128], BF16).ap()
    ulo = nc.dram_tensor("ulo", [(T + SL) * 16, 128], BF16).ap()

    # static per-step schedule
    # rh_max over a window (tau, tau+2^m]: widest consumer step
    def _rhmax(tau, m):
        return max(_fc(tp) for tp in range(tau + 1, tau + (1 << m) + 1))

    with tile.TileContext(nc) as tc:
        # ---------------- Stage A: U = X @ W_ih.T + bias ----------------
        with (
            tc.tile_pool(name="sa_w", bufs=1) as sa_w,
            tc.tile_pool(name="sa_x", bufs=3) as sa_x,
            tc.tile_pool(name="sa_u", bufs=4) as sa_u,
            tc.tile_pool(name="sa_ps", bufs=4, space="PSUM") as sa_ps,
        ):
            wih_sb = sa_w.tile([128, 8 * H], F32)  # 8 K-tiles of W_ih.T
            for c in range(8):
                nc.sync.dma_start(
                    wih_sb[:, c * H : (c + 1) * H], wihT[c * 128 : (c + 1) * 128, :]
                )
            bias_sb = sa_w.tile([128, H], F32)
            nc.sync.dma_start(bias_sb[:], bias_bc[:])

            uhi_v = uhi.rearrange("(t f) p -> t f p", f=16)
            ulo_v = ulo.rearrange("(t f) p -> t f p", f=16)

            for s in range(SS):
                xt_t = sa_x.tile([128, 8 * 128], F32, tag="xt")
                for c in range(8):
                    nc.sync.dma_start(
                        xt_t[:, c * 128 : (c + 1) * 128],
                        xT[c * 128 : (c + 1) * 128, s * SL : (s + 1) * SL],
                    )
                for n in range(4):  # r-chunks of 512
                    ps = sa_ps.tile([128, 512], F32, tag="aps")
                    for c in range(8):
                        nc.tensor.matmul(
                            ps[:],
                            xt_t[:, c * 128 : (c + 1) * 128],
                            wih_sb[:, c * H + n * 512 : c * H + (n + 1) * 512],
                            start=(c == 0),
                            stop=(c == 7),
                        )
                    uf = sa_u.tile([128, 512], F32, tag="uf")
                    nc.vector.tensor_add(
                        uf[:], ps[:], bias_sb[:, n * 512 : (n + 1) * 512]
                    )
                    uh = sa_u.tile([128, 512], BF16, tag="uh")
                    nc.scalar.activation(
                        uh[:], uf[:], mybir.ActivationFunctionType.Copy
                    )
                    ul = sa_u.tile([128, 512], BF16, tag="ul")
                    nc.vector.tensor_sub(ul[:], uf[:], uh[:])
                    dst_h = uhi_v[s * SL : (s + 1) * SL, 4 * n : 4 * n + 4, :]
                    dst_l = ulo_v[s * SL : (s + 1) * SL, 4 * n : 4 * n + 4, :]
                    src_h = uh[:].rearrange("t (f p) -> t f p", p=128)
                    src_l = ul[:].rearrange("t (f p) -> t f p", p=128)
                    nc.sync.dma_start(dst_h, src_h)
                    nc.sync.dma_start(dst_l, src_l)
                    if s == SS - 1:
                        # fill the prefetch-overrun pad with finite data
                        nc.sync.dma_start(
                            uhi_v[(s + 1) * SL : (s + 2) * SL, 4 * n : 4 * n + 4, :],
                            src_h,
                        )
                        nc.sync.dma_start(
                            ulo_v[(s + 1) * SL : (s + 2) * SL, 4 * n : 4 * n + 4, :],
                            src_l,
                        )

        # ---------------- Chain ----------------
        with (
            tc.tile_pool(name="ch_w", bufs=1) as ch_w,
            tc.tile_pool(name="ch_st", bufs=1) as ch_st,
            tc.tile_pool(name="ch_u", bufs=1) as ch_u,
            tc.tile_pool(name="ch_o", bufs=1) as ch_o,
            tc.tile_pool(name="ch_pre", bufs=4, space="PSUM") as ch_pre,
            tc.tile_pool(name="ch_pt", bufs=2, space="PSUM") as ch_pt,
        ):
            whh_sb = ch_w.tile([128, 16 * H], BF16)  # [q, c*2048 + rh*128 + p]
            for c in range(16):
                nc.sync.dma_start(
                    whh_sb[:, c * H : (c + 1) * H], whhT[c * 128 : (c + 1) * 128, :]
                )
            ident_sb = ch_w.tile([128, 128], F32)
            nc.sync.dma_start(ident_sb[:], ident[:])

            hbf = ch_st.tile([128, 16], BF16)
            nc.vector.memset(hbf[:], 0.0)

            # explicit ping-pong buffers (static parity inside the HW loop)
            ubufs = []
            for k in range(2):
                ubufs.append((
                    ch_u.tile([128, SL * 16], BF16, name=f"uhi{k}"),
                    ch_u.tile([128, SL * 16], BF16, name=f"ulo{k}"),
                    ch_u.tile([128, SL * 16], F32, name=f"u{k}"),
                ))
            obufs = []
            for k in range(2):
                obufs.append((
                    ch_o.tile([128, SL * 16], F32, name=f"osb{k}"),
                    ch_o.tile([128, H], F32, name=f"orow{k}"),
                ))

            def wtile(c, rh):
                return whh_sb[:, c * H + rh * 128 : c * H + (rh + 1) * 128]

            def prefetch_u_dma(k, pf):
                """Transpose-DMA superstep pf's U hi/lo into ubufs[k]."""
                uh, ul, _ = ubufs[k]
                nc.sync.dma_start(
                    uh[:], uhi[bass.ds(pf * (SL * 16), SL * 16), :], transpose=True
                )
                nc.sync.dma_start(
                    ul[:], ulo[bass.ds(pf * (SL * 16), SL * 16), :], transpose=True
                )

            def prefetch_u_add(k):
                uh, ul, uu = ubufs[k]
                nc.vector.tensor_add(uu[:], uh[:], ul[:])

            # Static emission schedule: every W_hh deposit is a chunk
            # (m, tau0, rh, c): module m updated at step tau0 (0 = superstep
            # carry-in), output row-half rh, contraction half c.  Its first
            # reader is tanh(tau0 + 2^(rh//2)) (or the single j-read in the
            # window for rh//2 > m), so the chunk is emitted in the PE stream
            # at slot deadline-1 — after that slot's tanh, ahead of the next
            # step's critical module-0 matmuls.  Slot 0 = before the step loop.
            emit_at = [[] for _ in range(SL)]  # slot tau: emitted after tanh(tau)
            for tau0 in range(0, SL):
                mmax = 7 if tau0 == 0 else _nu(tau0)
                m_lo = 0 if tau0 == 0 else 1
                for m in range(m_lo, mmax + 1):
                    rhm = _rhmax(tau0, m)
                    for rh in range(rhm):
                        j = rh // 2
                        if j <= m:
                            deadline = tau0 + (1 << j)
                        else:
                            p = 1 << j
                            deadline = ((tau0 // p) + 1) * p
                            if deadline > tau0 + (1 << m):
                                continue  # no j-read in window
                        for c in (2 * m, 2 * m + 1):
                            emit_at[deadline - 1].append((m, tau0, rh, c))

            def emit_superstep(k, s_expr, pf_expr, prefetch_next):
                """One 128-step superstep; ubufs[k] holds this superstep's U."""
                uu = ubufs[k][2]
                osb, orow = obufs[k]

                pre = [
                    ch_pre.tile([128, 512], F32, tag="pre", name=f"pre{k}_{b}")
                    for b in range(4)
                ]
                # U deposit opens each bank (overwrite), fp32 identity matmul
                for b in range(4):
                    if variant == "purecycle":
                        continue
                    # bank col layout is rh-major: col = rh*BK + (tau-1)%BK,
                    # u_sb is step-major: view as [p, f, t]
                    u_v = uu[:, 512 * b : 512 * (b + 1)].rearrange(
                        "p (t f) -> p f t", f=16
                    )
                    nc.tensor.matmul(
                        pre[b][:],
                        ident_sb[:],
                        u_v,
                        start=True,
                        stop=False,
                        skip_group_check=True,
                    )

                def emit_chunk(m, tau0, rh, c):
                    """Broadcast W.T[c-block, rh-block] @ h[c] into psum cols
                    {16*(tp-1)+rh : tp in (tau0, tau0+2^m]}, split per bank."""
                    n_w = 1 << m
                    tp0 = tau0 + 1
                    while tp0 <= tau0 + n_w:
                        b = (tp0 - 1) // BK
                        tp1 = min(tau0 + n_w, (b + 1) * BK)
                        n = tp1 - tp0 + 1
                        lt = (tp0 - 1) % BK
                        dst = pre[b][:, rh * BK + lt : rh * BK + lt + n]
                        nc.tensor.matmul(
                            dst,
                            wtile(c, rh),
                            hbf[:, c : c + 1].broadcast_to([128, n]),
                            start=False,
                            stop=False,
                            skip_group_check=True,
                        )
                        tp0 = tp1 + 1

                # slot 0: carry-in chunks due before tanh(1)
                if variant not in ("nodep", "purecycle"):
                    for (m, tau0, rh, c) in emit_at[0]:
                        emit_chunk(m, tau0, rh, c)

                # step loop
                for tau in range(1, SL + 1):
                    b = (tau - 1) // BK
                    lc = 16 * ((tau - 1) % BK)
                    fc = _fc(tau)
                    # module-0 deposit for THIS step (h_0 from tanh(tau-1))
                    lt = (tau - 1) % BK
                    if tau > 1:
                        for rh in range(fc):
                            for c in (0, 1):
                                nc.tensor.matmul(
                                    pre[b][:, rh * BK + lt : rh * BK + lt + 1],
                                    wtile(c, rh),
                                    hbf[:, c : c + 1],
                                    start=False,
                                    stop=False,
                                    skip_group_check=True,
                                )
                    nc.scalar.activation(
                        hbf[:, 0:fc],
                        pre[b][:].rearrange("p (f t) -> p t f", t=BK)[:, lt, 0:fc],
                        mybir.ActivationFunctionType.Tanh,
                    )
                    if variant != "purecycle":
                        nc.vector.tensor_copy(osb[:, lc + 512 * b : lc + 512 * b + 16], hbf[:])
                    # deferred deposit chunks due at this slot
                    if tau < SL and variant not in ("nodep", "purecycle"):
                        for (m, tau0, rh, c) in emit_at[tau]:
                            emit_chunk(m, tau0, rh, c)
                    if tau == 16 and prefetch_next and variant != "purecycle":
                        prefetch_u_dma(1 - k, pf_expr)
                    if tau == 48 and prefetch_next and variant != "purecycle":
                        prefetch_u_add(1 - k)

                # output: transpose [p, tau] -> [tau, p] per f, then DMA rows
                for f in range(16):
                    if variant == "purecycle":
                        break
                    tps = ch_pt.tile([128, 128], F32, tag="otp", name=f"otp{k}_{f}")
                    nc.tensor.transpose(
                        tps[:],
                        osb[:].rearrange("p (t f) -> p f t", f=16)[:, f, :],
                        ident_sb[:],
                    )
                    nc.vector.tensor_copy(orow[:, f * 128 : (f + 1) * 128], tps[:])
                if variant != "purecycle":
                    nc.sync.dma_start(out[bass.ds(s_expr * SL, SL), :], orow[:])

            # initial prefetch: superstep 0 into ubufs[0]
            prefetch_u_dma(0, 0)
            prefetch_u_add(0)

            with tc.For_i(
                0,
                SS * repeat // 2,
                1,
                hint_engines=(
                    mybir.EngineType.PE,
                    mybir.EngineType.Activation,
                    mybir.EngineType.DVE,
                ),
            ) as i_raw:
                for k in (0, 1):
                    if repeat == 1:
                        s_expr = 2 * i_raw + k
                        pf_expr = 2 * i_raw + k + 1
                    else:
                        s_expr = (2 * i_raw + k) % SS
                        pf_expr = (2 * i_raw + k + 1) % SS
                    emit_superstep(k, s_expr, pf_expr, prefetch_next=True)

    nc.compile()
    return nc


def _prep_inputs(x, W_ih, W_hh, b_ih, b_hh):
    return {
        "xT": np.ascontiguousarray(x.T).astype(np.float32),
        "wihT": np.ascontiguousarray(W_ih.T).astype(np.float32),
        "whhT": np.ascontiguousarray(W_hh.T).astype(ml_dtypes.bfloat16),
        "bias_bc": np.broadcast_to(
            (b_ih + b_hh).astype(np.float32), (128, H)
        ).copy(),
        "ident": np.eye(128, dtype=np.float32),
    }


_CACHE = {}


def _run(inputs, T=4096, trace=False):
    key = T
    if key not in _CACHE:
        _CACHE[key] = build_nc(T=T)
    nc = _CACHE[key]
    res = run_bass_kernel_spmd(nc, [inputs], [0], trace=trace)
    return res


def kernel(x, W_ih, W_hh, b_ih, b_hh):
    x = np.asarray(x, dtype=np.float32)
    T = x.shape[0]
    inputs = _prep_inputs(x, np.asarray(W_ih), np.asarray(W_hh), np.asarray(b_ih), np.asarray(b_hh))
    res = _run(inputs, T=T)
    return np.asarray(res.results[0]["out"], dtype=np.float32)



# revision 3
# speedup vs baseline: 1.0637x; 1.0637x over previous
"""Clockwork RNN (CwRNNCell) Trainium2 Bass kernel — v2.

Reference semantics (T=4096, H=2048, 8 modules of 256, periods 2^j):
  step t (1-indexed): module j active iff t % 2^j == 0
  pre = x_t @ W_ih.T + b_ih + h @ W_hh.T + b_hh
  h[active] = tanh(pre[active]);  inactive modules hold.

v2 vs baseline: Stage A (U = X@W_ih.T + bias) is folded into the chain loop
instead of running as a separate phase with a DRAM round-trip.  Per superstep,
U for the NEXT superstep is computed in row-major layout directly
(lhsT = W_ih.T tiles, rhs = x tiles -> PSUM [rows, steps]), bias-added into a
step-major bf16 SBUF buffer by DVE, and deposited into the chain's PSUM banks
by a single bf16 identity matmul per bank.  x / W_ih are bf16 (error budget
validated: ~1.1e-2 max rel err vs fp32 reference, gate 2e-2).

Chain (unchanged from baseline): per-step pre-activation tiles live in PSUM
(4 banks per 128-step superstep, 32 steps x 16 cols per bank) built by PE
accumulation; module-m updates deposit their W_hh column products into all
steps of their read window; module-0 products land per step.  Consume is one
ScalarE tanh per step; DVE stages h into the bf16 output buffer; outputs are
PE-transposed and DMA'd per superstep.
"""

import os

import numpy as np
import ml_dtypes

import concourse.bass as bass
import concourse.bacc as bacc
import concourse.mybir as mybir
from concourse import tile
from concourse.bass_utils import run_bass_kernel_spmd

BF16 = mybir.dt.bfloat16
F32 = mybir.dt.float32

H = 2048
IN = 1024
MS = 256
SL = 128  # superstep length (= max period)
BK = 32   # steps per psum bank (512 fp32 cols / 16)


def _ctz(t):
    return (t & -t).bit_length() - 1


def _nu(t):
    return min(_ctz(t), 7)


def _fc(t):
    return 2 * (_nu(t) + 1)


def build_nc(T=4096, num_cores=1, debug=False, enable_asserts=False, repeat=1):
    variant = os.environ.get("KV", "full")
    SS = T // SL
    assert (SS * repeat) % 2 == 0
    nc = bacc.Bacc(
        "TRN2",
        target_bir_lowering=False,
        debug=debug,
        enable_asserts=enable_asserts,
        num_devices=num_cores,
    )

    xTp = nc.dram_tensor("xTp", [IN, T + SL], BF16, kind="ExternalInput").ap()
    wihT = nc.dram_tensor("wihT", [IN, H], BF16, kind="ExternalInput").ap()
    whhT = nc.dram_tensor("whhT", [H, H], BF16, kind="ExternalInput").ap()
    bias_rm = nc.dram_tensor("bias_rm", [128, 16], F32, kind="ExternalInput").ap()
    identb = nc.dram_tensor("identb", [128, 128], BF16, kind="ExternalInput").ap()
    out = nc.dram_tensor("out", [T, H], F32, kind="ExternalOutput").ap()

    # static per-step schedule
    # rh_max over a window (tau, tau+2^m]: widest consumer step
    def _rhmax(tau, m):
        return max(_fc(tp) for tp in range(tau + 1, tau + (1 << m) + 1))

    # Static emission schedule: every W_hh deposit is a chunk (m, tau0, rh, c):
    # module m updated at step tau0 (0 = superstep carry-in), output row-half
    # rh, contraction half c.  It becomes emittable after tanh(tau0) (slot
    # tau0) and must execute before its first reader tanh(deadline) (slot
    # deadline-1).  Rather than emitting every chunk at deadline-1 (ALAP),
    # chunks with slack are list-scheduled across their feasible slot range to
    # flatten the per-slot PE load: odd steps have little module-0 work and
    # would otherwise idle the PE while high-fc steps oversubscribe it.
    # Slot 0 = before the step loop (carry-in chunks due before tanh(1)).
    chunks = []  # (ready_slot, last_slot, cost_instrs, (m, tau0, rh, c))
    for tau0 in range(0, SL):
        mmax = 7 if tau0 == 0 else _nu(tau0)
        m_lo = 0 if tau0 == 0 else 1
        for m in range(m_lo, mmax + 1):
            rhm = _rhmax(tau0, m)
            for rh in range(rhm):
                j = rh // 2
                if j <= m:
                    deadline = tau0 + (1 << j)
                else:
                    p = 1 << j
                    deadline = ((tau0 // p) + 1) * p
                    if deadline > tau0 + (1 << m):
                        continue  # no j-read in window
                # instruction count = number of bank splits
                n_w, tp0, nsplit = 1 << m, tau0 + 1, 0
                while tp0 <= tau0 + n_w:
                    b = (tp0 - 1) // BK
                    tp1 = min(tau0 + n_w, (b + 1) * BK)
                    nsplit += 1
                    tp0 = tp1 + 1
                for c in (2 * m, 2 * m + 1):
                    chunks.append((tau0, deadline - 1, nsplit, (m, tau0, rh, c)))

    # per-slot fixed load: module-0 deposits of the NEXT step (2*fc(tau+1)),
    # plus the U matmuls for the next superstep spread over slots 17..80
    load = [0.0] * SL
    for slot in range(SL):
        nxt = slot + 1
        if nxt <= SL:
            load[slot] += 2 * _fc(nxt) if nxt > 1 else 0
        if 17 <= slot <= 80:
            load[slot] += 2
    emit_at = [[] for _ in range(SL)]
    # zero-slack chunks first (forced), then by ascending slack
    chunks.sort(key=lambda ch: (ch[1] - ch[0], ch[1]))
    for ready, last, cost, key in chunks:
        best = min(range(ready, last + 1), key=lambda s: load[s])
        load[best] += cost
        emit_at[best].append((key, ready))
    # within a slot run stale chunks (ready < slot) first: they can execute
    # while this slot's tanh is still in flight on the Act engine
    for slot in range(SL):
        emit_at[slot].sort(key=lambda e: (e[1] >= slot, e[1]))
        emit_at[slot] = [key for key, _ in emit_at[slot]]

    with tile.TileContext(nc) as tc:
        with (
            tc.tile_pool(name="ch_w", bufs=1) as ch_w,
            tc.tile_pool(name="ch_st", bufs=1) as ch_st,
            tc.tile_pool(name="ch_x", bufs=2) as ch_x,
            tc.tile_pool(name="ch_u", bufs=1) as ch_u,
            tc.tile_pool(name="ch_o", bufs=1) as ch_o,
            tc.tile_pool(name="ch_pre", bufs=4, space="PSUM") as ch_pre,
            tc.tile_pool(name="ch_urs", bufs=2, space="PSUM") as ch_urs,
            tc.tile_pool(name="ch_pt", bufs=2, space="PSUM") as ch_pt,
        ):
            whh_sb = ch_w.tile([128, 16 * H], BF16)  # [q, c*2048 + rh*128 + p]
            for c in range(16):
                nc.sync.dma_start(
                    whh_sb[:, c * H : (c + 1) * H], whhT[c * 128 : (c + 1) * 128, :]
                )
            wih_sb = ch_w.tile([128, 8 * H], BF16)  # [k, c*2048 + r]
            for c in range(8):
                nc.sync.dma_start(
                    wih_sb[:, c * H : (c + 1) * H], wihT[c * 128 : (c + 1) * 128, :]
                )
            bias_sb = ch_w.tile([128, 16], F32)
            nc.sync.dma_start(bias_sb[:], bias_rm[:])
            identb_sb = ch_w.tile([128, 128], BF16)
            nc.sync.dma_start(identb_sb[:], identb[:])

            hbf = ch_st.tile([128, 16], BF16)
            nc.vector.memset(hbf[:], 0.0)

            # explicit ping-pong buffers (static parity inside the HW loop)
            # u_sb[k][p, 16*t + f] = U[row f*128+p, step t of superstep] + bias
            usb = [ch_u.tile([128, SL * 16], BF16, name=f"u{k}") for k in range(2)]
            obufs = []
            for k in range(2):
                obufs.append((
                    ch_o.tile([128, SL * 16], BF16, name=f"osb{k}"),
                    ch_o.tile([128, H], F32, name=f"orow{k}"),
                ))

            def wtile(c, rh):
                return whh_sb[:, c * H + rh * 128 : c * H + (rh + 1) * 128]

            def emit_xt_dma(xt_t, pf_expr):
                """DMA superstep pf's x columns into xt_t (bf16)."""
                for c in range(8):
                    nc.sync.dma_start(
                        xt_t[:, c * 128 : (c + 1) * 128],
                        xTp[c * 128 : (c + 1) * 128, bass.ds(pf_expr * SL, SL)],
                    )

            def emit_umm(state, xt_t, k_dst, pfx=""):
                """Emit one U matmul (row-major): state holds (idx, urs tile)."""
                idx = state[0]
                f, c = divmod(idx, 8)
                if c == 0:
                    state[1] = ch_urs.tile(
                        [128, 128], F32, tag="urs", name=f"urs{pfx}{k_dst}_{f}"
                    )
                urs = state[1]
                nc.tensor.matmul(
                    urs[:],
                    wih_sb[:, c * H + f * 128 : c * H + (f + 1) * 128],
                    xt_t[:, c * 128 : (c + 1) * 128],
                    start=(c == 0),
                    stop=(c == 7),
                )
                if c == 7:
                    # u_sb[k_dst] cols f, 16+f, ..., stride 16: [128, 128] view
                    dst = usb[k_dst][:].rearrange("p (t f) -> p f t", f=16)[:, f, :]
                    nc.vector.tensor_add(
                        dst, urs[:], bias_sb[:, f : f + 1].broadcast_to([128, SL])
                    )
                state[0] += 1

            def emit_superstep(k, s_expr, pf_expr, prefetch_next):
                """One 128-step superstep; usb[k] holds this superstep's U."""
                osb, orow = obufs[k]

                pre = [
                    ch_pre.tile([128, 512], F32, tag="pre", name=f"pre{k}_{b}")
                    for b in range(4)
                ]
                # U deposit opens each bank (overwrite), bf16 identity matmul
                for b in range(4):
                    if variant == "purecycle":
                        continue
                    # bank col layout is rh-major: col = rh*BK + (tau-1)%BK,
                    # u_sb is step-major: view as [p, f, t]
                    u_v = usb[k][:, 512 * b : 512 * (b + 1)].rearrange(
                        "p (t f) -> p f t", f=16
                    )
                    nc.tensor.matmul(
                        pre[b][:],
                        identb_sb[:],
                        u_v,
                        start=True,
                        stop=False,
                        skip_group_check=True,
                    )

                # next superstep's x tile + U-matmul emission state
                if prefetch_next and variant != "purecycle":
                    xt_t = ch_x.tile([128, 8 * 128], BF16, tag="xt")
                ustate = [0, None]

                def emit_chunk(m, tau0, rh, c):
                    """Broadcast W.T[c-block, rh-block] @ h[c] into psum cols
                    {16*(tp-1)+rh : tp in (tau0, tau0+2^m]}, split per bank."""
                    n_w = 1 << m
                    tp0 = tau0 + 1
                    while tp0 <= tau0 + n_w:
                        b = (tp0 - 1) // BK
                        tp1 = min(tau0 + n_w, (b + 1) * BK)
                        n = tp1 - tp0 + 1
                        lt = (tp0 - 1) % BK
                        dst = pre[b][:, rh * BK + lt : rh * BK + lt + n]
                        nc.tensor.matmul(
                            dst,
                            wtile(c, rh),
                            hbf[:, c : c + 1].broadcast_to([128, n]),
                            start=False,
                            stop=False,
                            skip_group_check=True,
                        )
                        tp0 = tp1 + 1

                # slot 0: carry-in chunks due before tanh(1)
                if variant not in ("nodep", "purecycle"):
                    for (m, tau0, rh, c) in emit_at[0]:
                        emit_chunk(m, tau0, rh, c)

                # step loop
                for tau in range(1, SL + 1):
                    b = (tau - 1) // BK
                    lc = 16 * ((tau - 1) % BK)
                    fc = _fc(tau)
                    # module-0 deposit for THIS step (h_0 from tanh(tau-1))
                    lt = (tau - 1) % BK
                    if tau > 1:
                        for rh in range(fc):
                            for c in (0, 1):
                                nc.tensor.matmul(
                                    pre[b][:, rh * BK + lt : rh * BK + lt + 1],
                                    wtile(c, rh),
                                    hbf[:, c : c + 1],
                                    start=False,
                                    stop=False,
                                    skip_group_check=True,
                                )
                    nc.scalar.activation(
                        hbf[:, 0:fc],
                        pre[b][:].rearrange("p (f t) -> p t f", t=BK)[:, lt, 0:fc],
                        mybir.ActivationFunctionType.Tanh,
                    )
                    if variant != "purecycle":
                        nc.vector.tensor_copy(
                            osb[:, lc + 512 * b : lc + 512 * b + 16], hbf[:]
                        )
                    # deferred deposit chunks due at this slot
                    if tau < SL and variant not in ("nodep", "purecycle"):
                        for (m, tau0, rh, c) in emit_at[tau]:
                            emit_chunk(m, tau0, rh, c)
                    if prefetch_next and variant != "purecycle":
                        if tau == 16:
                            emit_xt_dma(xt_t, pf_expr)
                        if 17 <= tau <= 80:
                            emit_umm(ustate, xt_t, 1 - k)
                            emit_umm(ustate, xt_t, 1 - k)

                # output: transpose [p, tau] -> [tau, p] per f, then DMA rows
                for f in range(16):
                    if variant == "purecycle":
                        break
                    tps = ch_pt.tile([128, 128], BF16, tag="otp", name=f"otp{k}_{f}")
                    nc.tensor.transpose(
                        tps[:],
                        osb[:].rearrange("p (t f) -> p f t", f=16)[:, f, :],
                        identb_sb[:],
                    )
                    nc.vector.tensor_copy(orow[:, f * 128 : (f + 1) * 128], tps[:])
                if variant != "purecycle":
                    nc.sync.dma_start(out[bass.ds(s_expr * SL, SL), :], orow[:])

            # initial U for superstep 0 into usb[0]
            if variant != "purecycle":
                xt0 = ch_x.tile([128, 8 * 128], BF16, tag="xt")
                emit_xt_dma(xt0, 0)
                st0 = [0, None]
                for _ in range(128):
                    emit_umm(st0, xt0, 0, pfx="i")

            with tc.For_i(
                0,
                SS * repeat // 2,
                1,
                hint_engines=(
                    mybir.EngineType.PE,
                    mybir.EngineType.Activation,
                    mybir.EngineType.DVE,
                ),
            ) as i_raw:
                for k in (0, 1):
                    if repeat == 1:
                        s_expr = 2 * i_raw + k
                        pf_expr = 2 * i_raw + k + 1
                    else:
                        s_expr = (2 * i_raw + k) % SS
                        pf_expr = (2 * i_raw + k + 1) % SS
                    emit_superstep(k, s_expr, pf_expr, prefetch_next=True)

    nc.compile()
    return nc


def _prep_inputs(x, W_ih, W_hh, b_ih, b_hh):
    T = x.shape[0]
    xT = np.ascontiguousarray(x.T).astype(ml_dtypes.bfloat16)
    xTp = np.concatenate([xT, np.zeros((IN, SL), ml_dtypes.bfloat16)], axis=1)
    bias = (np.asarray(b_ih) + np.asarray(b_hh)).astype(np.float32)
    return {
        "xTp": np.ascontiguousarray(xTp),
        "wihT": np.ascontiguousarray(np.asarray(W_ih).T).astype(ml_dtypes.bfloat16),
        "whhT": np.ascontiguousarray(np.asarray(W_hh).T).astype(ml_dtypes.bfloat16),
        "bias_rm": np.ascontiguousarray(bias.reshape(16, 128).T).astype(np.float32),
        "identb": np.eye(128, dtype=ml_dtypes.bfloat16),
    }


_CACHE = {}


def _get_exec(T):
    """Build + jit once per T; returns (jitted, in_names, out_names, zero_outs)."""
    if T in _CACHE:
        return _CACHE[T]
    import jax
    from concourse.bass2jax import (
        _bass_exec_p,
        install_neuronx_cc_hook,
        partition_id_tensor,
    )

    nc = build_nc(T=T)
    install_neuronx_cc_hook()
    in_names, out_names, out_avals, zero_outs = [], [], [], []
    for alloc in nc.m.functions[0].allocations:
        if not isinstance(alloc, mybir.MemoryLocationSet):
            continue
        name = alloc.memorylocations[0].name
        if alloc.kind == "ExternalInput":
            if nc.partition_id_tensor is None or name != nc.partition_id_tensor.name:
                in_names.append(name)
        elif alloc.kind == "ExternalOutput":
            shape = tuple(alloc.tensor_shape)
            dtype = mybir.dt.np(alloc.dtype)
            out_names.append(name)
            out_avals.append(jax.core.ShapedArray(shape, dtype))
            zero_outs.append(np.zeros(shape, dtype))
    all_names = list(in_names) + list(out_names)
    pname = nc.partition_id_tensor.name if nc.partition_id_tensor is not None else None
    if pname is not None:
        all_names = all_names + [pname]

    def _body(*args):
        operands = list(args)
        if pname is not None:
            operands.append(partition_id_tensor())
        outs = _bass_exec_p.bind(
            *operands,
            out_avals=tuple(out_avals),
            in_names=tuple(all_names),
            out_names=tuple(out_names),
            lowering_input_output_aliases=(),
            sim_require_finite=True,
            sim_require_nnan=True,
            nc=nc,
        )
        return tuple(outs)

    jitted = jax.jit(_body, keep_unused=True)
    _CACHE[T] = (jitted, in_names, out_names, zero_outs)
    return _CACHE[T]


def kernel(x, W_ih, W_hh, b_ih, b_hh):
    import jax

    x = np.asarray(x, dtype=np.float32)
    T = x.shape[0]
    inputs = _prep_inputs(
        x, np.asarray(W_ih), np.asarray(W_hh), np.asarray(b_ih), np.asarray(b_hh)
    )
    jitted, in_names, out_names, zero_outs = _get_exec(T)
    dev = jax.devices()[0]
    in_arrs = [jax.device_put(np.asarray(inputs[n]), dev) for n in in_names]
    zo = [jax.device_put(z, dev) for z in zero_outs]
    outs = jitted(*in_arrs, *zo)
    return np.asarray(outs[out_names.index("out")], dtype=np.float32)
